# revision 70
# baseline (speedup 1.0000x reference)
"""CopyDecoder Trainium2 kernel (nn_CopyDecoder_5274219840242).

Sharding: 8 cores = 4 batches x 2 query-halves (data parallel, no collectives).

Per core (b, q-slab of 256 rows):
  - attention: Q/K projections (fcQ folded into Wq on the host:
    Q = dec @ (Wq@WfcQ).T + (Wq@bfcQ + bq); computed transposed so the
    contraction dim lands on partitions; bf16 operands, fp32 accumulate),
    per-head softmax (logits bounded, so no max-subtraction), head mean.
  - duplicate-combining selection matrix Dm[s,s'] = [src_s == src_s'] built by
    compare-vs-transpose; a_comb = attn @ Dm gives each source position the
    full scatter-sum of its token; e = exp(a_comb/NH).
  - denom[q] = V + sum_s (e[q,s]-1)/cnt[s]  (softmax denominator over vocab,
    exploiting exp(0)=1 for vocab entries no source token maps to).
  - streaming blend over p1 in BF16 both directions (tolerance is 2e-2):
    out = (1-w)*p1 + w/denom.  Halves HBM traffic vs fp32 streaming, which
    is the roofline here (~99% DMA active in the fp32 baseline trace).
    The fused DVE tensor_scalar double-rounds (bf16 intermediate), which
    costs ~1.1e-2 rel err; instead each tile takes a single-rounding path:
    either one scalar-engine activation (out = Identity(p1*s1 + s2), fp32
    internal, per-partition scale/bias APs) or a DVE pair (mul to fp32
    intermediate, add to bf16).  Tiles are split between the two engines
    so neither becomes the bottleneck.
  - fix values for the <=512 source-token columns:
    fix[q,s] = (1-w)*p1[q,src_s] + (w/denom)*e[q,s]
    (p1 columns are host-gathered fp32 into an extra input; the host writes
    the fix columns into the final output during unshard).

Queue split (two HWDGE rings share 16 DMA engines, ~23.5GB/s each busy):
  - sync ring: Q-side weights (wqcb, decTb) first, then the pure p1 bf16
    load stream.
  - scalar ring: packed constants, K-side weights (wkb, encTb), decT,
    p1c, then all out-stores + fixc.
Weights ride ahead of the p1 stream on both rings so the attention chain
(which gates the first store via s2) starts ~5us in, not ~25us.  The
chain runs per q-partition-tile (mi) so the first blend stores start
while the second tile's softmax is still in flight.
"""

import sys

sys.path.insert(0, "/opt/trn_rl_repo")

import numpy as np

import concourse.bacc as bacc
import concourse.bass as bass
import concourse.mybir as mybir
import concourse.tile as tile
from concourse.bass_utils import run_bass_kernel_spmd
from concourse.masks import make_identity

P = 128
D = 512
TS = 512
TQH = 256  # q rows per core
V = 32000
NH = 8
DH = 64
KC = D // P  # 4 contraction chunks
MI = TQH // P  # 2 q partition tiles
SC = TS // P  # 4 source-position chunks
VT = 4000  # vocab columns per blend tile (8000B bf16 per partition row)
NVT = V // VT  # 8 vocab tiles per q partition tile

F32 = mybir.dt.float32
BF16 = mybir.dt.bfloat16
I32 = mybir.dt.int32
AF = mybir.ActivationFunctionType
ALU = mybir.AluOpType
AX = mybir.AxisListType

# packed per-partition constants layout (f32 columns):
#   [0:4) bqc   [4:8) bk   [8:12) wfcw   [12] -bfcw   [13] V-n_unique
PK = 14

_NC_CACHE = None
_LAST_RESULTS = None


def build_nc():
    nc = bacc.Bacc("TRN2", target_bir_lowering=False, debug=False)

    decT = nc.dram_tensor("decT", [D, TQH], F32, kind="ExternalInput")
    decTb = nc.dram_tensor("decTb", [D, TQH], BF16, kind="ExternalInput")
    encTb = nc.dram_tensor("encTb", [D, TS], BF16, kind="ExternalInput")
    wqcb = nc.dram_tensor("wqcb", [D, D], BF16, kind="ExternalInput")
    wkb = nc.dram_tensor("wkb", [D, D], BF16, kind="ExternalInput")
    pk = nc.dram_tensor("pk", [P, PK], F32, kind="ExternalInput")
    dmx = nc.dram_tensor("dmx", [P, SC * TS], BF16, kind="ExternalInput")
    p1 = nc.dram_tensor("p1", [TQH, V], BF16, kind="ExternalInput")
    p1c = nc.dram_tensor("p1c", [TQH, TS], BF16, kind="ExternalInput")
    out = nc.dram_tensor("out", [TQH, V], BF16, kind="ExternalOutput")
    fixc = nc.dram_tensor("fixc", [TQH, TS], F32, kind="ExternalOutput")

    with tile.TileContext(nc) as tc:
        with (
            tc.tile_pool(name="const", bufs=1) as cp,
            tc.tile_pool(name="work", bufs=6) as wp,
            tc.tile_pool(name="pin", bufs=12) as pinp,
            tc.tile_pool(name="pout", bufs=4) as poutp,
            tc.tile_pool(name="tmid", bufs=1) as tmidp,
            tc.tile_pool(name="ps", bufs=8, space="PSUM") as psp,
        ):
            # ---- persistent SBUF tiles ----
            decT_sb = cp.tile([P, KC, TQH], F32, tag="decT_sb")
            decTb_sb = cp.tile([P, KC, TQH], BF16, tag="decTb_sb")
            encTb_sb = cp.tile([P, KC, TS], BF16, tag="encTb_sb")
            wqcb_sb = cp.tile([P, KC, D], BF16, tag="wqcb_sb")
            wkb_sb = cp.tile([P, KC, D], BF16, tag="wkb_sb")
            pk_sb = cp.tile([P, PK], F32, tag="pk_sb")
            identb_sb = cp.tile([P, P], BF16, tag="identb_sb")
            Dm_sb = cp.tile([P, SC, TS], BF16, tag="Dm_sb")
            qTb_sb = cp.tile([P, KC, TQH], BF16, tag="qTb_sb")
            kTb_sb = cp.tile([P, KC, TS], BF16, tag="kTb_sb")
            attn_sb = cp.tile([P, MI, TS], BF16, tag="attn_sb")
            attnB_sb = cp.tile([P, MI, TS], BF16, tag="attnB_sb")
            attnT_sb = cp.tile([P, SC, TQH], BF16, tag="attnT_sb")
            e_sb = cp.tile([P, MI, TS], F32, tag="e_sb")
            p1c_sb = cp.tile([P, MI, TS], BF16, tag="p1c_sb")
            sume_sb = cp.tile([P, MI], F32, tag="sume_sb")
            denom_sb = cp.tile([P, MI], F32, tag="denom_sb")
            rden_sb = cp.tile([P, MI], F32, tag="rden_sb")
            w_sb = cp.tile([P, MI], F32, tag="w_sb")
            ez_sb = cp.tile([P, MI], F32, tag="ez_sb")
            t1_sb = cp.tile([P, MI], F32, tag="t1_sb")
            s1_sb = cp.tile([P, MI], F32, tag="s1_sb")
            s2_sb = cp.tile([P, MI], F32, tag="s2_sb")

            bqc_sb = pk_sb[:, 0:4]
            bk_sb = pk_sb[:, 4:8]
            wfcw_sb = pk_sb[:, 8:12]
            nbfcw_sb = pk_sb[:, 12:13]  # NEGATED gate bias (exp-form gate)
            vmu_sb = pk_sb[:, 13:14]  # V - n_unique(src)

            # ---- loads: Q-side operands lead the sync ring (ahead of the
            #      p1 stream); K-side operands + the rest lead the scalar
            #      ring (ahead of the out-stores).  First column-block of
            #      each weight goes separately so mc=0 matmuls start early.
            wqc_v = wqcb[:].rearrange("(c p) q -> p c q", p=P)
            wk_v = wkb[:].rearrange("(c p) q -> p c q", p=P)
            nc.sync.dma_start(
                out=decTb_sb[:], in_=decTb[:].rearrange("(c p) q -> p c q", p=P)
            )
            nc.sync.dma_start(out=wqcb_sb[:, :, 0:P], in_=wqc_v[:, :, 0:P])
            nc.sync.dma_start(out=wqcb_sb[:, :, P:D], in_=wqc_v[:, :, P:D])
            nc.sync.dma_start(
                out=decT_sb[:], in_=decT[:].rearrange("(c p) q -> p c q", p=P)
            )
            nc.scalar.dma_start(out=pk_sb[:], in_=pk[:])
            nc.scalar.dma_start(
                out=encTb_sb[:], in_=encTb[:].rearrange("(c p) q -> p c q", p=P)
            )
            nc.scalar.dma_start(out=wkb_sb[:, :, 0:P], in_=wk_v[:, :, 0:P])
            nc.scalar.dma_start(out=wkb_sb[:, :, P:D], in_=wk_v[:, :, P:D])
            nc.scalar.dma_start(
                out=Dm_sb[:], in_=dmx[:].rearrange("p (c s) -> p c s", c=SC)
            )
            nc.scalar.dma_start(
                out=p1c_sb[:], in_=p1c[:].rearrange("(mi p) s -> p mi s", p=P)
            )
            make_identity(nc, identb_sb[:])

            # pull the EXP activation table in off the critical path (the
            # scalar engine runs exclusively exps until the blend phase)
            junk = wp.tile([P, 1], F32, tag="junk")
            nc.scalar.activation(junk[:], pk_sb[:, 0:1], AF.Exp, bias=0.0, scale=1.0)

            # ---- per-chunk Q/K projections (bias-add on DVE, so the scalar
            #      engine stays on the exp table) interleaved with the scores
            #      + per-head softmax for the two heads living in that chunk:
            #      softmax pipelines with the projections and BOTH mi chains
            #      finish together ----
            for mc in range(KC):
                psq = psp.tile([P, TQH], F32, tag="ps")
                for kc in range(KC):
                    nc.tensor.matmul(
                        out=psq[:],
                        lhsT=wqcb_sb[:, kc, mc * P : (mc + 1) * P],
                        rhs=decTb_sb[:, kc, :],
                        start=(kc == 0),
                        stop=(kc == KC - 1),
                    )
                nc.vector.tensor_scalar_add(
                    qTb_sb[:, mc, :], psq[:], bqc_sb[:, mc : mc + 1]
                )
                psk = psp.tile([P, TS], F32, tag="ps")
                for kc in range(KC):
                    nc.tensor.matmul(
                        out=psk[:],
                        lhsT=wkb_sb[:, kc, mc * P : (mc + 1) * P],
                        rhs=encTb_sb[:, kc, :],
                        start=(kc == 0),
                        stop=(kc == KC - 1),
                    )
                nc.vector.tensor_scalar_add(
                    kTb_sb[:, mc, :], psk[:], bk_sb[:, mc : mc + 1]
                )
                # heads 2*mc and 2*mc+1 need only chunk mc of Q_T/K_T.
                # logits are ~N(0,1) so exp without max-subtraction is safe;
                # accumulate the sum of per-head softmaxes into TWO partial
                # chains per mi (halves the DVE dependency chain; combined
                # after the loop).  mi=0 first so its epilogue starts sooner.
                for mi in range(MI):
                    for hp in range(2):
                        tgt = attn_sb if hp == 0 else attnB_sb
                        ps = psp.tile([P, TS], F32, tag="ps")
                        nc.tensor.matmul(
                            out=ps[:],
                            lhsT=qTb_sb[hp * DH : (hp + 1) * DH, mc, mi * P : (mi + 1) * P],
                            rhs=kTb_sb[hp * DH : (hp + 1) * DH, mc, :],
                            start=True,
                            stop=True,
                        )
                        ex = wp.tile([P, TS], BF16, tag="ex")
                        se = wp.tile([P, 1], F32, tag="se")
                        nc.scalar.activation(
                            ex[:], ps[:], AF.Exp,
                            bias=0.0, scale=0.125, accum_out=se[:, 0:1],
                        )
                        r8 = wp.tile([P, 1], F32, tag="r8")
                        nc.vector.reciprocal(r8[:], se[:, 0:1])
                        if mc == 0:
                            nc.vector.tensor_scalar_mul(
                                tgt[:, mi, :], ex[:], r8[:, 0:1]
                            )
                        else:
                            nc.vector.scalar_tensor_tensor(
                                out=tgt[:, mi, :],
                                in0=ex[:],
                                scalar=r8[:, 0:1],
                                in1=tgt[:, mi, :],
                                op0=ALU.mult,
                                op1=ALU.add,
                            )
                if mc == 0:
                    # gate via the exp table (no sigmoid table switch):
                    # ez = exp(-(dec @ Wfcw.T + bfcw)); w = 1/(1+ez);
                    # s1 = 1-w = ez*w.  The matmul MUST be fp32: a bf16
                    # gate's worst-row error reaches ~1.3% on s1, which
                    # multiplies the whole p1 term (measured 1.44e-2).
                    for mi in range(MI):
                        psg = psp.tile([P, 1], F32, tag="ps")
                        for kc in range(KC):
                            nc.tensor.matmul(
                                out=psg[:],
                                lhsT=decT_sb[:, kc, mi * P : (mi + 1) * P],
                                rhs=wfcw_sb[:, kc : kc + 1],
                                start=(kc == 0),
                                stop=(kc == KC - 1),
                            )
                        nc.scalar.activation(
                            ez_sb[:, mi : mi + 1], psg[:], AF.Exp,
                            bias=nbfcw_sb[:, 0:1], scale=-1.0,
                        )
                    nc.vector.tensor_scalar_add(t1_sb[:], ez_sb[:], 1.0)
                    nc.vector.reciprocal(w_sb[:], t1_sb[:])
                    nc.vector.tensor_tensor(
                        out=s1_sb[:], in0=ez_sb[:], in1=w_sb[:], op=ALU.mult
                    )

            p1_v = p1[:].rearrange("(mi p) v -> p mi v", p=P)
            out_v = out[:].rearrange("(mi p) v -> p mi v", p=P)

            def epilogue(mi):
                # combine the two partial softmax sums, attn_T via PE
                # transpose, a_comb = attn@DmU (dedup columns: one per
                # unique token, zero-padded), e = exp(a_comb/NH) whose
                # accumulator directly yields the softmax denominator:
                # padding columns contribute exp(0)=1 each, so
                # denom = accum + (V - TS).
                nc.vector.tensor_tensor(
                    out=attn_sb[:, mi, :], in0=attn_sb[:, mi, :],
                    in1=attnB_sb[:, mi, :], op=ALU.add,
                )
                for sc in range(SC):
                    pt = psp.tile([P, P], BF16, tag="ps")
                    nc.tensor.transpose(
                        out=pt[:],
                        in_=attn_sb[:, mi, sc * P : (sc + 1) * P],
                        identity=identb_sb[:],
                    )
                    nc.vector.tensor_copy(attnT_sb[:, sc, mi * P : (mi + 1) * P], pt[:])
                ps = psp.tile([P, TS], F32, tag="ps")
                for c in range(SC):
                    nc.tensor.matmul(
                        out=ps[:],
                        lhsT=attnT_sb[:, c, mi * P : (mi + 1) * P],
                        rhs=Dm_sb[:, c, :],
                        start=(c == 0),
                        stop=(c == SC - 1),
                    )
                nc.scalar.activation(
                    e_sb[:, mi, :], ps[:], AF.Exp, bias=0.0, scale=1.0 / NH,
                    accum_out=sume_sb[:, mi : mi + 1],
                )
                nc.vector.tensor_scalar_add(
                    denom_sb[:, mi : mi + 1], sume_sb[:, mi : mi + 1],
                    float(V - TS),
                )
                nc.vector.reciprocal(rden_sb[:, mi : mi + 1], denom_sb[:, mi : mi + 1])
                nc.vector.tensor_tensor(
                    out=s2_sb[:, mi : mi + 1], in0=w_sb[:, mi : mi + 1],
                    in1=rden_sb[:, mi : mi + 1], op=ALU.mult,
                )

            def blend_tile(mi, vt, path, defer_store=False):
                vs = slice(vt * VT, (vt + 1) * VT)
                pin = pinp.tile([P, VT], BF16, tag="pin")
                nc.sync.dma_start(out=pin[:], in_=p1_v[:, mi, vs])
                pout = poutp.tile([P, VT], BF16, tag="pout")
                if path == "act":
                    # one scalar-engine op, fp32 internal, single bf16 round
                    nc.scalar.activation(
                        pout[:], pin[:], AF.Identity,
                        bias=s2_sb[:, mi : mi + 1],
                        scale=s1_sb[:, mi : mi + 1],
                    )
                else:
                    # DVE pair with fp32 intermediate: also a single bf16 round
                    t = tmidp.tile([P, VT], F32, tag="tmid")
                    nc.vector.tensor_scalar_mul(
                        t[:], pin[:], s1_sb[:, mi : mi + 1]
                    )
                    nc.vector.tensor_scalar_add(
                        pout[:], t[:], s2_sb[:, mi : mi + 1]
                    )
                if not defer_store:
                    nc.scalar.dma_start(out=out_v[:, mi, vs], in_=pout[:])
                return pout

            # both epilogues before any act-blend: the scalar engine then
            # switches from the exp table to identity exactly once
            epilogue(0)
            epilogue(1)

            # fix columns (dedup order): fix = s1*p1c + s2*e, written back
            # into e_sb; the host scatters fixc[:, uidx] into the output
            for mi in range(MI):
                t2 = wp.tile([P, TS], F32, tag="fix_t2")
                nc.vector.tensor_scalar_mul(t2[:], e_sb[:, mi, :], s2_sb[:, mi : mi + 1])
                nc.vector.scalar_tensor_tensor(
                    out=e_sb[:, mi, :],
                    in0=p1c_sb[:, mi, :],
                    scalar=s1_sb[:, mi : mi + 1],
                    op0=ALU.mult,
                    in1=t2[:],
                    op1=ALU.add,
                )
            nc.scalar.dma_start(
                out=fixc[:].rearrange("(mi p) s -> p mi s", p=P), in_=e_sb[:]
            )

            tiles = [(m, v) for m in range(MI) for v in range(NVT)]
            for i, (mi, vt) in enumerate(tiles):
                blend_tile(mi, vt, "dve" if i % 2 == 0 else "act")

    nc.finalize()
    return nc


def _get_nc():
    global _NC_CACHE
    if _NC_CACHE is None:
        _NC_CACHE = build_nc()
    return _NC_CACHE


def kernel(**inputs) -> np.ndarray:
    dec = np.asarray(inputs["dec_output"], dtype=np.float32)  # [4, 512, 512]
    enc = np.asarray(inputs["enc_output"], dtype=np.float32)  # [4, 512, 512]
    src = np.asarray(inputs["src"]).astype(np.int32)  # [4, 512]
    p1 = np.asarray(inputs["p1"], dtype=np.float32)  # [4, 512, 32000]
    WfcQ = np.asarray(inputs["WfcQ"], dtype=np.float32)
    bfcQ = np.asarray(inputs["bfcQ"], dtype=np.float32)
    Wq = np.asarray(inputs["Wq"], dtype=np.float32)
    bq = np.asarray(inputs["bq"], dtype=np.float32)
    Wk = np.asarray(inputs["Wk"], dtype=np.float32)
    bk = np.asarray(inputs["bk"], dtype=np.float32)
    Wfcw = np.asarray(inputs["Wfcw"], dtype=np.float32)
    bfcw = np.asarray(inputs["bfcw"], dtype=np.float32)

    B, TQ, _ = dec.shape
    n_cores = 8

    import ml_dtypes

    bf16 = ml_dtypes.bfloat16
    # fold fcQ into the query projection (cq feeds nothing else)
    Wqc = Wq @ WfcQ
    bqc = Wq @ bfcQ + bq
    wqcb = np.ascontiguousarray(Wqc.T.astype(bf16))
    wkb = np.ascontiguousarray(Wk.T.astype(bf16))

    in_maps = []
    uidx_by_core = []
    for core in range(n_cores):
        b, qh = core // 2, core % 2
        qs = slice(qh * TQH, (qh + 1) * TQH)
        p1_slab = p1[b, qs, :]
        # packed per-partition constants: [p, c] = x[c*128 + p]
        pk = np.zeros((P, PK), np.float32)
        pk[:, 0:4] = bqc.reshape(KC, P).T
        pk[:, 4:8] = bk.reshape(KC, P).T
        pk[:, 8:12] = Wfcw[0].reshape(KC, P).T
        pk[:, 12] = -bfcw[0]  # negated: gate uses exp(-(z + bfcw))
        # dedup scatter matrix: one column per unique token (zero-padded);
        # the e-exp accumulator then directly yields the softmax denominator
        tok, uidx = np.unique(src[b], return_inverse=True)
        DmU = np.zeros((TS, TS), np.float32)
        DmU[np.arange(TS), uidx] = 1.0  # [s, u]
        dmx = np.ascontiguousarray(
            DmU.reshape(SC, P, TS).transpose(1, 0, 2).reshape(P, SC * TS).astype(bf16)
        )
        uidx_by_core.append(uidx)
        p1cp = np.zeros((TQH, TS), np.float32)
        p1cp[:, : tok.size] = p1_slab[:, tok]
        in_maps.append(
            {
                "decT": np.ascontiguousarray(dec[b].T[:, qs]),
                "decTb": np.ascontiguousarray(dec[b].T[:, qs].astype(bf16)),
                "encTb": np.ascontiguousarray(enc[b].T.astype(bf16)),
                "wqcb": wqcb,
                "wkb": wkb,
                "pk": pk,
                "dmx": dmx,
                "p1": np.ascontiguousarray(p1_slab.astype(bf16)),
                "p1c": np.ascontiguousarray(p1cp.astype(bf16)),
            }
        )

    nc = _get_nc()
    res = run_bass_kernel_spmd(nc, in_maps, core_ids=list(range(n_cores)))
    global _LAST_RESULTS
    _LAST_RESULTS = res

    out = np.empty((B, TQ, V), dtype=np.float32)
    for core in range(n_cores):
        b, qh = core // 2, core % 2
        qs = slice(qh * TQH, (qh + 1) * TQH)
        out[b, qs, :] = res.results[core]["out"].astype(np.float32)
        # place the corrected source-token columns (duplicates carry
        # identical values, so overwrite order does not matter)
        out[b, qs, :][:, src[b]] = res.results[core]["fixc"][:, uidx_by_core[core]]
    return out


# revision 78
# speedup vs baseline: 1.1395x; 1.1395x over previous
"""CopyDecoder Trainium2 kernel (nn_CopyDecoder_5274219840242).

Sharding: 8 cores = 4 batches x 2 query-halves (data parallel, no collectives).

Per core (b, q-slab of 256 rows):
  - attention: Q/K projections (fcQ folded into Wq on the host:
    Q = dec @ (Wq@WfcQ).T + (Wq@bfcQ + bq); computed transposed so the
    contraction dim lands on partitions; bf16 operands, fp32 accumulate),
    per-head softmax (logits bounded, so no max-subtraction), head mean.
  - duplicate-combining selection matrix Dm[s,s'] = [src_s == src_s'] built by
    compare-vs-transpose; a_comb = attn @ Dm gives each source position the
    full scatter-sum of its token; e = exp(a_comb/NH).
  - denom[q] = V + sum_s (e[q,s]-1)/cnt[s]  (softmax denominator over vocab,
    exploiting exp(0)=1 for vocab entries no source token maps to).
  - streaming blend over p1 in BF16 both directions (tolerance is 2e-2):
    out = (1-w)*p1 + w/denom.  Halves HBM traffic vs fp32 streaming, which
    is the roofline here (~99% DMA active in the fp32 baseline trace).
    The fused DVE tensor_scalar double-rounds (bf16 intermediate), which
    costs ~1.1e-2 rel err; instead each tile takes a single-rounding path:
    either one scalar-engine activation (out = Identity(p1*s1 + s2), fp32
    internal, per-partition scale/bias APs) or a DVE pair (mul to fp32
    intermediate, add to bf16).  Tiles are split between the two engines
    so neither becomes the bottleneck.
  - fix values for the <=512 source-token columns:
    fix[q,s] = (1-w)*p1[q,src_s] + (w/denom)*e[q,s]
    (p1 columns are host-gathered fp32 into an extra input; the host writes
    the fix columns into the final output during unshard).

Queue split (two HWDGE rings share 16 DMA engines, ~23.5GB/s each busy):
  - sync ring: Q-side weights (wqcb, decTb) first, then the pure p1 bf16
    load stream.
  - scalar ring: packed constants, K-side weights (wkb, encTb), decT,
    p1c, then all out-stores + fixc.
Weights ride ahead of the p1 stream on both rings so the attention chain
(which gates the first store via s2) starts ~5us in, not ~25us.  The
chain runs per q-partition-tile (mi) so the first blend stores start
while the second tile's softmax is still in flight.
"""

import sys

sys.path.insert(0, "/opt/trn_rl_repo")

import numpy as np

import concourse.bacc as bacc
import concourse.bass as bass
import concourse.mybir as mybir
import concourse.tile as tile
from concourse.bass_utils import run_bass_kernel_spmd
from concourse.masks import make_identity

P = 128
D = 512
TS = 512
TQH = 256  # q rows per core
V = 32000
NH = 8
DH = 64
KC = D // P  # 4 contraction chunks
MI = TQH // P  # 2 q partition tiles
SC = TS // P  # 4 source-position chunks
VT = 4000  # vocab columns per blend tile (8000B bf16 per partition row)
NVT = V // VT  # 8 vocab tiles per q partition tile

F32 = mybir.dt.float32
BF16 = mybir.dt.bfloat16
I32 = mybir.dt.int32
AF = mybir.ActivationFunctionType
ALU = mybir.AluOpType
AX = mybir.AxisListType

# packed per-partition constants layout (f32 columns):
#   [0:4) bqc   [4:8) bk   [8:12) wfcw   [12] -bfcw   [13] V-n_unique
PK = 14

_NC_CACHE = None
_LAST_RESULTS = None


def build_nc():
    nc = bacc.Bacc("TRN2", target_bir_lowering=False, debug=False)

    decT = nc.dram_tensor("decT", [D, TQH], F32, kind="ExternalInput")
    decTb = nc.dram_tensor("decTb", [D, TQH], BF16, kind="ExternalInput")
    encTb = nc.dram_tensor("encTb", [D, TS], BF16, kind="ExternalInput")
    wqcb = nc.dram_tensor("wqcb", [D, D], BF16, kind="ExternalInput")
    wkb = nc.dram_tensor("wkb", [D, D], BF16, kind="ExternalInput")
    pk = nc.dram_tensor("pk", [P, PK], F32, kind="ExternalInput")
    dmx = nc.dram_tensor("dmx", [P, SC * TS], BF16, kind="ExternalInput")
    p1 = nc.dram_tensor("p1", [TQH, V], BF16, kind="ExternalInput")
    p1c = nc.dram_tensor("p1c", [TQH, TS], BF16, kind="ExternalInput")
    out = nc.dram_tensor("out", [TQH, V], BF16, kind="ExternalOutput")
    fixc = nc.dram_tensor("fixc", [TQH, TS], BF16, kind="ExternalOutput")

    with tile.TileContext(nc) as tc:
        with (
            tc.tile_pool(name="const", bufs=1) as cp,
            tc.tile_pool(name="work", bufs=6) as wp,
            tc.tile_pool(name="pin", bufs=12) as pinp,
            tc.tile_pool(name="pout", bufs=4) as poutp,
            tc.tile_pool(name="tmid", bufs=1) as tmidp,
            tc.tile_pool(name="ps", bufs=8, space="PSUM") as psp,
        ):
            # ---- persistent SBUF tiles ----
            decT_sb = cp.tile([P, KC, TQH], F32, tag="decT_sb")
            decTb_sb = cp.tile([P, KC, TQH], BF16, tag="decTb_sb")
            encTb_sb = cp.tile([P, KC, TS], BF16, tag="encTb_sb")
            wqcb_sb = cp.tile([P, KC, D], BF16, tag="wqcb_sb")
            wkb_sb = cp.tile([P, KC, D], BF16, tag="wkb_sb")
            pk_sb = cp.tile([P, PK], F32, tag="pk_sb")
            identb_sb = cp.tile([P, P], BF16, tag="identb_sb")
            Dm_sb = cp.tile([P, SC, TS], BF16, tag="Dm_sb")
            qTb_sb = cp.tile([P, KC, TQH], BF16, tag="qTb_sb")
            kTb_sb = cp.tile([P, KC, TS], BF16, tag="kTb_sb")
            attn_sb = cp.tile([P, MI, TS], BF16, tag="attn_sb")
            attnB_sb = cp.tile([P, MI, TS], BF16, tag="attnB_sb")
            attnT_sb = cp.tile([P, SC, TQH], BF16, tag="attnT_sb")
            e_sb = cp.tile([P, MI, TS], F32, tag="e_sb")
            fixb_sb = cp.tile([P, MI, TS], BF16, tag="fixb_sb")
            p1c_sb = cp.tile([P, MI, TS], BF16, tag="p1c_sb")
            sume_sb = cp.tile([P, MI], F32, tag="sume_sb")
            denom_sb = cp.tile([P, MI], F32, tag="denom_sb")
            rden_sb = cp.tile([P, MI], F32, tag="rden_sb")
            w_sb = cp.tile([P, MI], F32, tag="w_sb")
            ez_sb = cp.tile([P, MI], F32, tag="ez_sb")
            t1_sb = cp.tile([P, MI], F32, tag="t1_sb")
            s1_sb = cp.tile([P, MI], F32, tag="s1_sb")
            s2_sb = cp.tile([P, MI], F32, tag="s2_sb")

            bqc_sb = pk_sb[:, 0:4]
            bk_sb = pk_sb[:, 4:8]
            wfcw_sb = pk_sb[:, 8:12]
            nbfcw_sb = pk_sb[:, 12:13]  # NEGATED gate bias (exp-form gate)
            vmu_sb = pk_sb[:, 13:14]  # V - n_unique(src)

            # ---- loads: Q-side operands lead the sync ring (ahead of the
            #      p1 stream); K-side operands + the rest lead the scalar
            #      ring (ahead of the out-stores).  First column-block of
            #      each weight goes separately so mc=0 matmuls start early.
            wqc_v = wqcb[:].rearrange("(c p) q -> p c q", p=P)
            wk_v = wkb[:].rearrange("(c p) q -> p c q", p=P)
            nc.sync.dma_start(
                out=decTb_sb[:], in_=decTb[:].rearrange("(c p) q -> p c q", p=P)
            )
            nc.sync.dma_start(out=wqcb_sb[:, :, 0:P], in_=wqc_v[:, :, 0:P])
            nc.sync.dma_start(out=wqcb_sb[:, :, P:D], in_=wqc_v[:, :, P:D])
            nc.sync.dma_start(
                out=decT_sb[:], in_=decT[:].rearrange("(c p) q -> p c q", p=P)
            )
            nc.scalar.dma_start(out=pk_sb[:], in_=pk[:])
            nc.scalar.dma_start(
                out=encTb_sb[:], in_=encTb[:].rearrange("(c p) q -> p c q", p=P)
            )
            nc.scalar.dma_start(out=wkb_sb[:, :, 0:P], in_=wk_v[:, :, 0:P])
            nc.scalar.dma_start(out=wkb_sb[:, :, P:D], in_=wk_v[:, :, P:D])
            nc.scalar.dma_start(
                out=Dm_sb[:], in_=dmx[:].rearrange("p (c s) -> p c s", c=SC)
            )
            nc.scalar.dma_start(
                out=p1c_sb[:], in_=p1c[:].rearrange("(mi p) s -> p mi s", p=P)
            )
            make_identity(nc, identb_sb[:])

            # pull the EXP activation table in off the critical path (the
            # scalar engine runs exclusively exps until the blend phase)
            junk = wp.tile([P, 1], F32, tag="junk")
            nc.scalar.activation(junk[:], pk_sb[:, 0:1], AF.Exp, bias=0.0, scale=1.0)

            def head_softmax(mc, hp, mi):
                tgt = attn_sb if hp == 0 else attnB_sb
                ps = psp.tile([P, TS], F32, tag="ps")
                nc.tensor.matmul(
                    out=ps[:],
                    lhsT=qTb_sb[hp * DH : (hp + 1) * DH, mc, mi * P : (mi + 1) * P],
                    rhs=kTb_sb[hp * DH : (hp + 1) * DH, mc, :],
                    start=True,
                    stop=True,
                )
                ex = wp.tile([P, TS], BF16, tag="ex")
                se = wp.tile([P, 1], F32, tag="se")
                nc.scalar.activation(
                    ex[:], ps[:], AF.Exp,
                    bias=0.0, scale=0.125, accum_out=se[:, 0:1],
                )
                r8 = wp.tile([P, 1], F32, tag="r8")
                nc.vector.reciprocal(r8[:], se[:, 0:1])
                if mc == 0:
                    nc.vector.tensor_scalar_mul(tgt[:, mi, :], ex[:], r8[:, 0:1])
                else:
                    nc.vector.scalar_tensor_tensor(
                        out=tgt[:, mi, :],
                        in0=ex[:],
                        scalar=r8[:, 0:1],
                        in1=tgt[:, mi, :],
                        op0=ALU.mult,
                        op1=ALU.add,
                    )

            # ---- per-chunk Q/K projections (bias-add on DVE, so the scalar
            #      engine stays on the exp table) interleaved with the scores
            #      + per-head softmax for the two heads living in that chunk:
            #      softmax pipelines with the projections and BOTH mi chains
            #      finish together ----
            for mc in range(KC):
                psq = psp.tile([P, TQH], F32, tag="ps")
                for kc in range(KC):
                    nc.tensor.matmul(
                        out=psq[:],
                        lhsT=wqcb_sb[:, kc, mc * P : (mc + 1) * P],
                        rhs=decTb_sb[:, kc, :],
                        start=(kc == 0),
                        stop=(kc == KC - 1),
                    )
                nc.vector.tensor_scalar_add(
                    qTb_sb[:, mc, :], psq[:], bqc_sb[:, mc : mc + 1]
                )
                psk = psp.tile([P, TS], F32, tag="ps")
                for kc in range(KC):
                    nc.tensor.matmul(
                        out=psk[:],
                        lhsT=wkb_sb[:, kc, mc * P : (mc + 1) * P],
                        rhs=encTb_sb[:, kc, :],
                        start=(kc == 0),
                        stop=(kc == KC - 1),
                    )
                nc.vector.tensor_scalar_add(
                    kTb_sb[:, mc, :], psk[:], bk_sb[:, mc : mc + 1]
                )
                # heads 2*mc and 2*mc+1 need only chunk mc of Q_T/K_T.
                # logits are ~N(0,1) so exp without max-subtraction is safe;
                # accumulate the sum of per-head softmaxes into TWO partial
                # chains per mi (halves the DVE dependency chain; combined
                # in the epilogue).  Only mi=0 scores run inside this loop
                # (pipelined under the projections) so its epilogue — which
                # gates the first store — finishes ~8us sooner; mi=1 runs
                # right after.
                for hp in range(2):
                    head_softmax(mc, hp, 0)
                if mc == 0:
                    # gate via the exp table (no sigmoid table switch):
                    # ez = exp(-(dec @ Wfcw.T + bfcw)); w = 1/(1+ez);
                    # s1 = 1-w = ez*w.  The matmul MUST be fp32: a bf16
                    # gate's worst-row error reaches ~1.3% on s1, which
                    # multiplies the whole p1 term (measured 1.44e-2).
                    for mi in range(MI):
                        psg = psp.tile([P, 1], F32, tag="ps")
                        for kc in range(KC):
                            nc.tensor.matmul(
                                out=psg[:],
                                lhsT=decT_sb[:, kc, mi * P : (mi + 1) * P],
                                rhs=wfcw_sb[:, kc : kc + 1],
                                start=(kc == 0),
                                stop=(kc == KC - 1),
                            )
                        nc.scalar.activation(
                            ez_sb[:, mi : mi + 1], psg[:], AF.Exp,
                            bias=nbfcw_sb[:, 0:1], scale=-1.0,
                        )
                    nc.vector.tensor_scalar_add(t1_sb[:], ez_sb[:], 1.0)
                    nc.vector.reciprocal(w_sb[:], t1_sb[:])
                    nc.vector.tensor_tensor(
                        out=s1_sb[:], in0=ez_sb[:], in1=w_sb[:], op=ALU.mult
                    )

            p1_v = p1[:].rearrange("(mi p) v -> p mi v", p=P)
            out_v = out[:].rearrange("(mi p) v -> p mi v", p=P)

            def epilogue(mi):
                # combine the two partial softmax sums, attn_T via PE
                # transpose, a_comb = attn@DmU (dedup columns: one per
                # unique token, zero-padded), e = exp(a_comb/NH) whose
                # accumulator directly yields the softmax denominator:
                # padding columns contribute exp(0)=1 each, so
                # denom = accum + (V - TS).
                nc.vector.tensor_tensor(
                    out=attn_sb[:, mi, :], in0=attn_sb[:, mi, :],
                    in1=attnB_sb[:, mi, :], op=ALU.add,
                )
                for sc in range(SC):
                    pt = psp.tile([P, P], BF16, tag="ps")
                    nc.tensor.transpose(
                        out=pt[:],
                        in_=attn_sb[:, mi, sc * P : (sc + 1) * P],
                        identity=identb_sb[:],
                    )
                    nc.vector.tensor_copy(attnT_sb[:, sc, mi * P : (mi + 1) * P], pt[:])
                ps = psp.tile([P, TS], F32, tag="ps")
                for c in range(SC):
                    nc.tensor.matmul(
                        out=ps[:],
                        lhsT=attnT_sb[:, c, mi * P : (mi + 1) * P],
                        rhs=Dm_sb[:, c, :],
                        start=(c == 0),
                        stop=(c == SC - 1),
                    )
                nc.scalar.activation(
                    e_sb[:, mi, :], ps[:], AF.Exp, bias=0.0, scale=1.0 / NH,
                    accum_out=sume_sb[:, mi : mi + 1],
                )
                nc.vector.tensor_scalar_add(
                    denom_sb[:, mi : mi + 1], sume_sb[:, mi : mi + 1],
                    float(V - TS),
                )
                nc.vector.reciprocal(rden_sb[:, mi : mi + 1], denom_sb[:, mi : mi + 1])
                nc.vector.tensor_tensor(
                    out=s2_sb[:, mi : mi + 1], in0=w_sb[:, mi : mi + 1],
                    in1=rden_sb[:, mi : mi + 1], op=ALU.mult,
                )

            def blend_tile(mi, vt, path, defer_store=False):
                vs = slice(vt * VT, (vt + 1) * VT)
                pin = pinp.tile([P, VT], BF16, tag="pin")
                nc.sync.dma_start(out=pin[:], in_=p1_v[:, mi, vs])
                pout = poutp.tile([P, VT], BF16, tag="pout")
                if path == "act":
                    # one scalar-engine op, fp32 internal, single bf16 round
                    nc.scalar.activation(
                        pout[:], pin[:], AF.Identity,
                        bias=s2_sb[:, mi : mi + 1],
                        scale=s1_sb[:, mi : mi + 1],
                    )
                else:
                    # DVE pair with fp32 intermediate: also a single bf16 round
                    t = tmidp.tile([P, VT], F32, tag="tmid")
                    nc.vector.tensor_scalar_mul(
                        t[:], pin[:], s1_sb[:, mi : mi + 1]
                    )
                    nc.vector.tensor_scalar_add(
                        pout[:], t[:], s2_sb[:, mi : mi + 1]
                    )
                if not defer_store:
                    nc.scalar.dma_start(out=out_v[:, mi, vs], in_=pout[:])
                return pout

            # mi=0 epilogue first -> one DVE blend starts the store stream
            # early; then mi=1 scores/softmax (PE is free, exps stay on the
            # exp table), its epilogue, fix, and the remaining tiles.
            epilogue(0)
            blend_tile(0, 0, "dve")
            for mc in range(KC):
                for hp in range(2):
                    head_softmax(mc, hp, 1)
            epilogue(1)

            # fix columns (dedup order): fix = s1*p1c + s2*e (bf16 out);
            # the host scatters fixc[:, uidx] into the output
            for mi in range(MI):
                t2 = wp.tile([P, TS], F32, tag="fix_t2")
                nc.vector.tensor_scalar_mul(t2[:], e_sb[:, mi, :], s2_sb[:, mi : mi + 1])
                nc.vector.scalar_tensor_tensor(
                    out=fixb_sb[:, mi, :],
                    in0=p1c_sb[:, mi, :],
                    scalar=s1_sb[:, mi : mi + 1],
                    op0=ALU.mult,
                    in1=t2[:],
                    op1=ALU.add,
                )
            nc.scalar.dma_start(
                out=fixc[:].rearrange("(mi p) s -> p mi s", p=P), in_=fixb_sb[:]
            )

            tiles = [(0, v) for v in range(1, NVT)] + [(1, v) for v in range(NVT)]
            for i, (mi, vt) in enumerate(tiles):
                blend_tile(mi, vt, "act" if i % 2 == 0 else "dve")

    nc.finalize()
    return nc


def _get_nc():
    global _NC_CACHE
    if _NC_CACHE is None:
        _NC_CACHE = build_nc()
    return _NC_CACHE


def kernel(**inputs) -> np.ndarray:
    dec = np.asarray(inputs["dec_output"], dtype=np.float32)  # [4, 512, 512]
    enc = np.asarray(inputs["enc_output"], dtype=np.float32)  # [4, 512, 512]
    src = np.asarray(inputs["src"]).astype(np.int32)  # [4, 512]
    p1 = np.asarray(inputs["p1"], dtype=np.float32)  # [4, 512, 32000]
    WfcQ = np.asarray(inputs["WfcQ"], dtype=np.float32)
    bfcQ = np.asarray(inputs["bfcQ"], dtype=np.float32)
    Wq = np.asarray(inputs["Wq"], dtype=np.float32)
    bq = np.asarray(inputs["bq"], dtype=np.float32)
    Wk = np.asarray(inputs["Wk"], dtype=np.float32)
    bk = np.asarray(inputs["bk"], dtype=np.float32)
    Wfcw = np.asarray(inputs["Wfcw"], dtype=np.float32)
    bfcw = np.asarray(inputs["bfcw"], dtype=np.float32)

    B, TQ, _ = dec.shape
    n_cores = 8

    import ml_dtypes

    bf16 = ml_dtypes.bfloat16
    # fold fcQ into the query projection (cq feeds nothing else)
    Wqc = Wq @ WfcQ
    bqc = Wq @ bfcQ + bq
    wqcb = np.ascontiguousarray(Wqc.T.astype(bf16))
    wkb = np.ascontiguousarray(Wk.T.astype(bf16))

    in_maps = []
    uidx_by_core = []
    for core in range(n_cores):
        b, qh = core // 2, core % 2
        qs = slice(qh * TQH, (qh + 1) * TQH)
        p1_slab = p1[b, qs, :]
        # packed per-partition constants: [p, c] = x[c*128 + p]
        pk = np.zeros((P, PK), np.float32)
        pk[:, 0:4] = bqc.reshape(KC, P).T
        pk[:, 4:8] = bk.reshape(KC, P).T
        pk[:, 8:12] = Wfcw[0].reshape(KC, P).T
        pk[:, 12] = -bfcw[0]  # negated: gate uses exp(-(z + bfcw))
        # dedup scatter matrix: one column per unique token (zero-padded);
        # the e-exp accumulator then directly yields the softmax denominator
        tok, uidx = np.unique(src[b], return_inverse=True)
        DmU = np.zeros((TS, TS), np.float32)
        DmU[np.arange(TS), uidx] = 1.0  # [s, u]
        dmx = np.ascontiguousarray(
            DmU.reshape(SC, P, TS).transpose(1, 0, 2).reshape(P, SC * TS).astype(bf16)
        )
        uidx_by_core.append(uidx)
        p1cp = np.zeros((TQH, TS), np.float32)
        p1cp[:, : tok.size] = p1_slab[:, tok]
        in_maps.append(
            {
                "decT": np.ascontiguousarray(dec[b].T[:, qs]),
                "decTb": np.ascontiguousarray(dec[b].T[:, qs].astype(bf16)),
                "encTb": np.ascontiguousarray(enc[b].T.astype(bf16)),
                "wqcb": wqcb,
                "wkb": wkb,
                "pk": pk,
                "dmx": dmx,
                "p1": np.ascontiguousarray(p1_slab.astype(bf16)),
                "p1c": np.ascontiguousarray(p1cp.astype(bf16)),
            }
        )

    nc = _get_nc()
    res = run_bass_kernel_spmd(nc, in_maps, core_ids=list(range(n_cores)))
    global _LAST_RESULTS
    _LAST_RESULTS = res

    out = np.empty((B, TQ, V), dtype=np.float32)
    for core in range(n_cores):
        b, qh = core // 2, core % 2
        qs = slice(qh * TQH, (qh + 1) * TQH)
        out[b, qs, :] = res.results[core]["out"].astype(np.float32)
        # place the corrected source-token columns (duplicates carry
        # identical values, so overwrite order does not matter)
        out[b, qs, :][:, src[b]] = (
            res.results[core]["fixc"].astype(np.float32)[:, uidx_by_core[core]]
        )
    return out


# revision 80
# speedup vs baseline: 1.1701x; 1.0268x over previous
"""CopyDecoder Trainium2 kernel (nn_CopyDecoder_5274219840242).

Sharding: 8 cores = 4 batches x 2 query-halves (data parallel, no collectives).

Per core (b, q-slab of 256 rows):
  - attention: Q/K projections (fcQ folded into Wq on the host:
    Q = dec @ (Wq@WfcQ).T + (Wq@bfcQ + bq); computed transposed so the
    contraction dim lands on partitions; bf16 operands, fp32 accumulate),
    per-head softmax (logits bounded, so no max-subtraction), head mean.
  - duplicate-combining selection matrix Dm[s,s'] = [src_s == src_s'] built by
    compare-vs-transpose; a_comb = attn @ Dm gives each source position the
    full scatter-sum of its token; e = exp(a_comb/NH).
  - denom[q] = V + sum_s (e[q,s]-1)/cnt[s]  (softmax denominator over vocab,
    exploiting exp(0)=1 for vocab entries no source token maps to).
  - streaming blend over p1 in BF16 both directions (tolerance is 2e-2):
    out = (1-w)*p1 + w/denom.  Halves HBM traffic vs fp32 streaming, which
    is the roofline here (~99% DMA active in the fp32 baseline trace).
    The fused DVE tensor_scalar double-rounds (bf16 intermediate), which
    costs ~1.1e-2 rel err; instead each tile takes a single-rounding path:
    either one scalar-engine activation (out = Identity(p1*s1 + s2), fp32
    internal, per-partition scale/bias APs) or a DVE pair (mul to fp32
    intermediate, add to bf16).  Tiles are split between the two engines
    so neither becomes the bottleneck.
  - fix values for the <=512 source-token columns:
    fix[q,s] = (1-w)*p1[q,src_s] + (w/denom)*e[q,s]
    (p1 columns are host-gathered fp32 into an extra input; the host writes
    the fix columns into the final output during unshard).

Queue split (two HWDGE rings share 16 DMA engines, ~23.5GB/s each busy):
  - sync ring: Q-side weights (wqcb, decTb) first, then the pure p1 bf16
    load stream.
  - scalar ring: packed constants, K-side weights (wkb, encTb), decT,
    p1c, then all out-stores + fixc.
Weights ride ahead of the p1 stream on both rings so the attention chain
(which gates the first store via s2) starts ~5us in, not ~25us.  The
chain runs per q-partition-tile (mi) so the first blend stores start
while the second tile's softmax is still in flight.
"""

import sys

sys.path.insert(0, "/opt/trn_rl_repo")

import numpy as np

import concourse.bacc as bacc
import concourse.bass as bass
import concourse.mybir as mybir
import concourse.tile as tile
from concourse.bass_utils import run_bass_kernel_spmd
from concourse.masks import make_identity

P = 128
D = 512
TS = 512
TQH = 256  # q rows per core
V = 32000
NH = 8
DH = 64
KC = D // P  # 4 contraction chunks
MI = TQH // P  # 2 q partition tiles
SC = TS // P  # 4 source-position chunks
VT = 4000  # vocab columns per blend tile (8000B bf16 per partition row)
NVT = V // VT  # 8 vocab tiles per q partition tile

F32 = mybir.dt.float32
BF16 = mybir.dt.bfloat16
I32 = mybir.dt.int32
AF = mybir.ActivationFunctionType
ALU = mybir.AluOpType
AX = mybir.AxisListType

# packed per-partition constants layout (f32 columns):
#   [0:4) bqc   [4:8) bk   [8:12) wfcw   [12] -bfcw   [13] V-n_unique
PK = 14

_NC_CACHE = None
_LAST_RESULTS = None


def build_nc():
    nc = bacc.Bacc("TRN2", target_bir_lowering=False, debug=False)

    decT = nc.dram_tensor("decT", [D, TQH], F32, kind="ExternalInput")
    decTb = nc.dram_tensor("decTb", [D, TQH], BF16, kind="ExternalInput")
    encTb = nc.dram_tensor("encTb", [D, TS], BF16, kind="ExternalInput")
    wqcb = nc.dram_tensor("wqcb", [D, D], BF16, kind="ExternalInput")
    wkb = nc.dram_tensor("wkb", [D, D], BF16, kind="ExternalInput")
    pk = nc.dram_tensor("pk", [P, PK], F32, kind="ExternalInput")
    dmx = nc.dram_tensor("dmx", [P, SC * TS], BF16, kind="ExternalInput")
    p1 = nc.dram_tensor("p1", [TQH, V], BF16, kind="ExternalInput")
    p1c = nc.dram_tensor("p1c", [TQH, TS], BF16, kind="ExternalInput")
    out = nc.dram_tensor("out", [TQH, V], BF16, kind="ExternalOutput")
    fixc = nc.dram_tensor("fixc", [TQH, TS], BF16, kind="ExternalOutput")

    with tile.TileContext(nc) as tc:
        with (
            tc.tile_pool(name="const", bufs=1) as cp,
            tc.tile_pool(name="work", bufs=6) as wp,
            tc.tile_pool(name="pin", bufs=12) as pinp,
            tc.tile_pool(name="pout", bufs=4) as poutp,
            tc.tile_pool(name="tmid", bufs=1) as tmidp,
            tc.tile_pool(name="ps", bufs=8, space="PSUM") as psp,
        ):
            # ---- persistent SBUF tiles ----
            decT_sb = cp.tile([P, KC, TQH], F32, tag="decT_sb")
            decTb_sb = cp.tile([P, KC, TQH], BF16, tag="decTb_sb")
            encTb_sb = cp.tile([P, KC, TS], BF16, tag="encTb_sb")
            wqcb_sb = cp.tile([P, KC, D], BF16, tag="wqcb_sb")
            wkb_sb = cp.tile([P, KC, D], BF16, tag="wkb_sb")
            pk_sb = cp.tile([P, PK], F32, tag="pk_sb")
            identb_sb = cp.tile([P, P], BF16, tag="identb_sb")
            Dm_sb = cp.tile([P, SC, TS], BF16, tag="Dm_sb")
            qTb_sb = cp.tile([P, KC, TQH], BF16, tag="qTb_sb")
            kTb_sb = cp.tile([P, KC, TS], BF16, tag="kTb_sb")
            attn_sb = cp.tile([P, MI, TS], BF16, tag="attn_sb")
            attnB_sb = cp.tile([P, MI, TS], BF16, tag="attnB_sb")
            attnT_sb = cp.tile([P, SC, TQH], BF16, tag="attnT_sb")
            e_sb = cp.tile([P, MI, TS], F32, tag="e_sb")
            fixb_sb = cp.tile([P, MI, TS], BF16, tag="fixb_sb")
            p1c_sb = cp.tile([P, MI, TS], BF16, tag="p1c_sb")
            sume_sb = cp.tile([P, MI], F32, tag="sume_sb")
            denom_sb = cp.tile([P, MI], F32, tag="denom_sb")
            rden_sb = cp.tile([P, MI], F32, tag="rden_sb")
            w_sb = cp.tile([P, MI], F32, tag="w_sb")
            ez_sb = cp.tile([P, MI], F32, tag="ez_sb")
            t1_sb = cp.tile([P, MI], F32, tag="t1_sb")
            s1_sb = cp.tile([P, MI], F32, tag="s1_sb")
            s2_sb = cp.tile([P, MI], F32, tag="s2_sb")

            bqc_sb = pk_sb[:, 0:4]
            bk_sb = pk_sb[:, 4:8]
            wfcw_sb = pk_sb[:, 8:12]
            nbfcw_sb = pk_sb[:, 12:13]  # NEGATED gate bias (exp-form gate)
            vmu_sb = pk_sb[:, 13:14]  # V - n_unique(src)

            # ---- loads: Q-side operands lead the sync ring (ahead of the
            #      p1 stream); K-side operands + the rest lead the scalar
            #      ring (ahead of the out-stores).  First column-block of
            #      each weight goes separately so mc=0 matmuls start early.
            wqc_v = wqcb[:].rearrange("(c p) q -> p c q", p=P)
            wk_v = wkb[:].rearrange("(c p) q -> p c q", p=P)
            nc.sync.dma_start(
                out=decTb_sb[:], in_=decTb[:].rearrange("(c p) q -> p c q", p=P)
            )
            nc.sync.dma_start(out=wqcb_sb[:, :, 0:P], in_=wqc_v[:, :, 0:P])
            nc.sync.dma_start(out=wqcb_sb[:, :, P:D], in_=wqc_v[:, :, P:D])
            nc.sync.dma_start(
                out=decT_sb[:], in_=decT[:].rearrange("(c p) q -> p c q", p=P)
            )
            nc.scalar.dma_start(out=pk_sb[:], in_=pk[:])
            nc.scalar.dma_start(
                out=encTb_sb[:], in_=encTb[:].rearrange("(c p) q -> p c q", p=P)
            )
            nc.scalar.dma_start(out=wkb_sb[:, :, 0:P], in_=wk_v[:, :, 0:P])
            nc.scalar.dma_start(out=wkb_sb[:, :, P:D], in_=wk_v[:, :, P:D])
            nc.scalar.dma_start(
                out=Dm_sb[:], in_=dmx[:].rearrange("p (c s) -> p c s", c=SC)
            )
            nc.scalar.dma_start(
                out=p1c_sb[:], in_=p1c[:].rearrange("(mi p) s -> p mi s", p=P)
            )
            make_identity(nc, identb_sb[:])

            # pull the EXP activation table in off the critical path (the
            # scalar engine runs exclusively exps until the blend phase)
            junk = wp.tile([P, 1], F32, tag="junk")
            nc.scalar.activation(junk[:], pk_sb[:, 0:1], AF.Exp, bias=0.0, scale=1.0)

            def head_softmax(mc, hp, mi):
                tgt = attn_sb if hp == 0 else attnB_sb
                ps = psp.tile([P, TS], F32, tag="ps")
                nc.tensor.matmul(
                    out=ps[:],
                    lhsT=qTb_sb[hp * DH : (hp + 1) * DH, mc, mi * P : (mi + 1) * P],
                    rhs=kTb_sb[hp * DH : (hp + 1) * DH, mc, :],
                    start=True,
                    stop=True,
                )
                ex = wp.tile([P, TS], BF16, tag="ex")
                se = wp.tile([P, 1], F32, tag="se")
                nc.scalar.activation(
                    ex[:], ps[:], AF.Exp,
                    bias=0.0, scale=0.125, accum_out=se[:, 0:1],
                )
                r8 = wp.tile([P, 1], F32, tag="r8")
                nc.vector.reciprocal(r8[:], se[:, 0:1])
                if mc == 0:
                    nc.vector.tensor_scalar_mul(tgt[:, mi, :], ex[:], r8[:, 0:1])
                else:
                    nc.vector.scalar_tensor_tensor(
                        out=tgt[:, mi, :],
                        in0=ex[:],
                        scalar=r8[:, 0:1],
                        in1=tgt[:, mi, :],
                        op0=ALU.mult,
                        op1=ALU.add,
                    )

            # ---- per-chunk Q/K projections (bias-add on DVE, so the scalar
            #      engine stays on the exp table) interleaved with the scores
            #      + per-head softmax for the two heads living in that chunk:
            #      softmax pipelines with the projections and BOTH mi chains
            #      finish together ----
            for mc in range(KC):
                psq = psp.tile([P, TQH], F32, tag="ps")
                for kc in range(KC):
                    nc.tensor.matmul(
                        out=psq[:],
                        lhsT=wqcb_sb[:, kc, mc * P : (mc + 1) * P],
                        rhs=decTb_sb[:, kc, :],
                        start=(kc == 0),
                        stop=(kc == KC - 1),
                    )
                nc.vector.tensor_scalar_add(
                    qTb_sb[:, mc, :], psq[:], bqc_sb[:, mc : mc + 1]
                )
                psk = psp.tile([P, TS], F32, tag="ps")
                for kc in range(KC):
                    nc.tensor.matmul(
                        out=psk[:],
                        lhsT=wkb_sb[:, kc, mc * P : (mc + 1) * P],
                        rhs=encTb_sb[:, kc, :],
                        start=(kc == 0),
                        stop=(kc == KC - 1),
                    )
                nc.vector.tensor_scalar_add(
                    kTb_sb[:, mc, :], psk[:], bk_sb[:, mc : mc + 1]
                )
                # heads 2*mc and 2*mc+1 need only chunk mc of Q_T/K_T.
                # logits are ~N(0,1) so exp without max-subtraction is safe;
                # accumulate the sum of per-head softmaxes into TWO partial
                # chains per mi (halves the DVE dependency chain; combined
                # in the epilogue).  Only mi=0 scores run inside this loop
                # (pipelined under the projections) so its epilogue — which
                # gates the first store — finishes ~8us sooner; mi=1 runs
                # right after.
                for hp in range(2):
                    head_softmax(mc, hp, 0)
                if mc == 0:
                    # gate via the exp table (no sigmoid table switch):
                    # ez = exp(-(dec @ Wfcw.T + bfcw)); w = 1/(1+ez);
                    # s1 = 1-w = ez*w.  The matmul MUST be fp32: a bf16
                    # gate's worst-row error reaches ~1.3% on s1, which
                    # multiplies the whole p1 term (measured 1.44e-2).
                    for mi in range(MI):
                        psg = psp.tile([P, 1], F32, tag="ps")
                        for kc in range(KC):
                            nc.tensor.matmul(
                                out=psg[:],
                                lhsT=decT_sb[:, kc, mi * P : (mi + 1) * P],
                                rhs=wfcw_sb[:, kc : kc + 1],
                                start=(kc == 0),
                                stop=(kc == KC - 1),
                            )
                        nc.scalar.activation(
                            ez_sb[:, mi : mi + 1], psg[:], AF.Exp,
                            bias=nbfcw_sb[:, 0:1], scale=-1.0,
                        )
                    nc.vector.tensor_scalar_add(t1_sb[:], ez_sb[:], 1.0)
                    nc.vector.reciprocal(w_sb[:], t1_sb[:])
                    nc.vector.tensor_tensor(
                        out=s1_sb[:], in0=ez_sb[:], in1=w_sb[:], op=ALU.mult
                    )

            p1_v = p1[:].rearrange("(mi p) v -> p mi v", p=P)
            out_v = out[:].rearrange("(mi p) v -> p mi v", p=P)

            def epilogue(mi):
                # combine the two partial softmax sums, attn_T via PE
                # transpose, a_comb = attn@DmU (dedup columns: one per
                # unique token, zero-padded), e = exp(a_comb/NH) whose
                # accumulator directly yields the softmax denominator:
                # padding columns contribute exp(0)=1 each, so
                # denom = accum + (V - TS).
                nc.vector.tensor_tensor(
                    out=attn_sb[:, mi, :], in0=attn_sb[:, mi, :],
                    in1=attnB_sb[:, mi, :], op=ALU.add,
                )
                for sc in range(SC):
                    pt = psp.tile([P, P], BF16, tag="ps")
                    nc.tensor.transpose(
                        out=pt[:],
                        in_=attn_sb[:, mi, sc * P : (sc + 1) * P],
                        identity=identb_sb[:],
                    )
                    nc.vector.tensor_copy(attnT_sb[:, sc, mi * P : (mi + 1) * P], pt[:])
                ps = psp.tile([P, TS], F32, tag="ps")
                for c in range(SC):
                    nc.tensor.matmul(
                        out=ps[:],
                        lhsT=attnT_sb[:, c, mi * P : (mi + 1) * P],
                        rhs=Dm_sb[:, c, :],
                        start=(c == 0),
                        stop=(c == SC - 1),
                    )
                nc.scalar.activation(
                    e_sb[:, mi, :], ps[:], AF.Exp, bias=0.0, scale=1.0 / NH,
                    accum_out=sume_sb[:, mi : mi + 1],
                )
                nc.vector.tensor_scalar_add(
                    denom_sb[:, mi : mi + 1], sume_sb[:, mi : mi + 1],
                    float(V - TS),
                )
                nc.vector.reciprocal(rden_sb[:, mi : mi + 1], denom_sb[:, mi : mi + 1])
                nc.vector.tensor_tensor(
                    out=s2_sb[:, mi : mi + 1], in0=w_sb[:, mi : mi + 1],
                    in1=rden_sb[:, mi : mi + 1], op=ALU.mult,
                )

            def blend_tile(mi, vt, path, defer_store=False):
                vs = slice(vt * VT, (vt + 1) * VT)
                pin = pinp.tile([P, VT], BF16, tag="pin")
                nc.sync.dma_start(out=pin[:], in_=p1_v[:, mi, vs])
                pout = poutp.tile([P, VT], BF16, tag="pout")
                if path == "act":
                    # one scalar-engine op, fp32 internal, single bf16 round
                    nc.scalar.activation(
                        pout[:], pin[:], AF.Identity,
                        bias=s2_sb[:, mi : mi + 1],
                        scale=s1_sb[:, mi : mi + 1],
                    )
                else:
                    # DVE pair with fp32 intermediate: also a single bf16 round
                    t = tmidp.tile([P, VT], F32, tag="tmid")
                    nc.vector.tensor_scalar_mul(
                        t[:], pin[:], s1_sb[:, mi : mi + 1]
                    )
                    nc.vector.tensor_scalar_add(
                        pout[:], t[:], s2_sb[:, mi : mi + 1]
                    )
                if not defer_store:
                    nc.scalar.dma_start(out=out_v[:, mi, vs], in_=pout[:])
                return pout

            # mi=0 epilogue first, then keep the store stream fed while the
            # mi=1 chain runs: two act blends right after (costs two extra
            # activation-table switches, cheaper than a 10us store bubble)
            # and dve blends woven between mi=1's softmax accumulations.
            epilogue(0)
            blend_tile(0, 0, "dve")
            blend_tile(0, 1, "act")
            blend_tile(0, 2, "act")
            head_softmax(0, 0, 1)
            head_softmax(0, 1, 1)
            blend_tile(0, 3, "dve")
            head_softmax(1, 0, 1)
            head_softmax(1, 1, 1)
            head_softmax(2, 0, 1)
            head_softmax(2, 1, 1)
            blend_tile(0, 4, "dve")
            head_softmax(3, 0, 1)
            head_softmax(3, 1, 1)
            epilogue(1)

            # fix columns (dedup order): fix = s1*p1c + s2*e (bf16 out);
            # the host scatters fixc[:, uidx] into the output
            for mi in range(MI):
                t2 = wp.tile([P, TS], F32, tag="fix_t2")
                nc.vector.tensor_scalar_mul(t2[:], e_sb[:, mi, :], s2_sb[:, mi : mi + 1])
                nc.vector.scalar_tensor_tensor(
                    out=fixb_sb[:, mi, :],
                    in0=p1c_sb[:, mi, :],
                    scalar=s1_sb[:, mi : mi + 1],
                    op0=ALU.mult,
                    in1=t2[:],
                    op1=ALU.add,
                )
            nc.scalar.dma_start(
                out=fixc[:].rearrange("(mi p) s -> p mi s", p=P), in_=fixb_sb[:]
            )

            tiles = [(0, v) for v in range(5, NVT)] + [(1, v) for v in range(NVT)]
            for i, (mi, vt) in enumerate(tiles):
                blend_tile(mi, vt, "act" if i % 2 == 0 else "dve")

    nc.finalize()
    return nc


def _get_nc():
    global _NC_CACHE
    if _NC_CACHE is None:
        _NC_CACHE = build_nc()
    return _NC_CACHE


def kernel(**inputs) -> np.ndarray:
    dec = np.asarray(inputs["dec_output"], dtype=np.float32)  # [4, 512, 512]
    enc = np.asarray(inputs["enc_output"], dtype=np.float32)  # [4, 512, 512]
    src = np.asarray(inputs["src"]).astype(np.int32)  # [4, 512]
    p1 = np.asarray(inputs["p1"], dtype=np.float32)  # [4, 512, 32000]
    WfcQ = np.asarray(inputs["WfcQ"], dtype=np.float32)
    bfcQ = np.asarray(inputs["bfcQ"], dtype=np.float32)
    Wq = np.asarray(inputs["Wq"], dtype=np.float32)
    bq = np.asarray(inputs["bq"], dtype=np.float32)
    Wk = np.asarray(inputs["Wk"], dtype=np.float32)
    bk = np.asarray(inputs["bk"], dtype=np.float32)
    Wfcw = np.asarray(inputs["Wfcw"], dtype=np.float32)
    bfcw = np.asarray(inputs["bfcw"], dtype=np.float32)

    B, TQ, _ = dec.shape
    n_cores = 8

    import ml_dtypes

    bf16 = ml_dtypes.bfloat16
    # fold fcQ into the query projection (cq feeds nothing else)
    Wqc = Wq @ WfcQ
    bqc = Wq @ bfcQ + bq
    wqcb = np.ascontiguousarray(Wqc.T.astype(bf16))
    wkb = np.ascontiguousarray(Wk.T.astype(bf16))

    in_maps = []
    uidx_by_core = []
    for core in range(n_cores):
        b, qh = core // 2, core % 2
        qs = slice(qh * TQH, (qh + 1) * TQH)
        p1_slab = p1[b, qs, :]
        # packed per-partition constants: [p, c] = x[c*128 + p]
        pk = np.zeros((P, PK), np.float32)
        pk[:, 0:4] = bqc.reshape(KC, P).T
        pk[:, 4:8] = bk.reshape(KC, P).T
        pk[:, 8:12] = Wfcw[0].reshape(KC, P).T
        pk[:, 12] = -bfcw[0]  # negated: gate uses exp(-(z + bfcw))
        # dedup scatter matrix: one column per unique token (zero-padded);
        # the e-exp accumulator then directly yields the softmax denominator
        tok, uidx = np.unique(src[b], return_inverse=True)
        DmU = np.zeros((TS, TS), np.float32)
        DmU[np.arange(TS), uidx] = 1.0  # [s, u]
        dmx = np.ascontiguousarray(
            DmU.reshape(SC, P, TS).transpose(1, 0, 2).reshape(P, SC * TS).astype(bf16)
        )
        uidx_by_core.append(uidx)
        p1cp = np.zeros((TQH, TS), np.float32)
        p1cp[:, : tok.size] = p1_slab[:, tok]
        in_maps.append(
            {
                "decT": np.ascontiguousarray(dec[b].T[:, qs]),
                "decTb": np.ascontiguousarray(dec[b].T[:, qs].astype(bf16)),
                "encTb": np.ascontiguousarray(enc[b].T.astype(bf16)),
                "wqcb": wqcb,
                "wkb": wkb,
                "pk": pk,
                "dmx": dmx,
                "p1": np.ascontiguousarray(p1_slab.astype(bf16)),
                "p1c": np.ascontiguousarray(p1cp.astype(bf16)),
            }
        )

    nc = _get_nc()
    res = run_bass_kernel_spmd(nc, in_maps, core_ids=list(range(n_cores)))
    global _LAST_RESULTS
    _LAST_RESULTS = res

    out = np.empty((B, TQ, V), dtype=np.float32)
    for core in range(n_cores):
        b, qh = core // 2, core % 2
        qs = slice(qh * TQH, (qh + 1) * TQH)
        out[b, qs, :] = res.results[core]["out"].astype(np.float32)
        # place the corrected source-token columns (duplicates carry
        # identical values, so overwrite order does not matter)
        out[b, qs, :][:, src[b]] = (
            res.results[core]["fixc"].astype(np.float32)[:, uidx_by_core[core]]
        )
    return out
